# revision 11
# baseline (speedup 1.0000x reference)
"""Trainium2 Bass kernel for nn_LinkPredictor.

Reference computation (B=4, N=256, T=16, F=128, H=256):
    h = mean_T(nodefeat)                      # [B,N,F]
    a = h @ W1[:, :F].T                       # [B,N,H]
    c = h @ W1[:, F:].T                       # [B,N,H]
    logits[b,i,j] = W2[0] . relu(a[b,i] + c[b,j] + b1) + b2   # [B,N,N]

Sharding: 8 cores; core k handles batch b=k//2, i-half k%2 (128 i-rows x
256 j-cols of one batch's NxN grid).  Each core only needs nodefeat[b]
(2.1 MB), since the pairwise grid never mixes batches.

Per-core device plan (all layouts h/f-on-partitions):
  - hT[f, j] via 32 PE matmuls: stationary = nodefeat octet [(j8,t16)=128p, f],
    moving = selection matrix S [128, 8] with S[(j8,t), m] = (j8==m)/16.
  - cT/aT via fp32 matmuls with W1 slices as stationary; b1 folded into aT.
  - Pairwise: for each i (128) and h-tile (2): act = relu(cT + aT[:,i])
    on VectorE (tensor_scalar add+max, 2x fp32) or ScalarE (activation
    Relu with per-partition bias), split ~2:1 for engine balance.
  - Reduction over h on TensorE: act is the moving operand (float32r,
    1 cyc/row at 256 cols); stationary = [128, 32] zero-padded w2 column
    r=i%32 placed at col-group c=i//32, PSUM-accumulating all 128 i-rows
    into a single [128 i, 256 j] PSUM tile.  One tensor_scalar (+b2)
    drains it to SBUF, one DMA out.
"""

import os
import sys

import numpy as np

_B, _N, _T, _F, _H = 4, 256, 16, 128, 256
_NCORES = 8

_CACHE = {}


def _ensure_paths():
    for p in (
        "/root/.axon_site",
        "/root/.axon_site/_ro/trn_rl_repo",
        "/root/.axon_site/_ro/pypackages",
        "/opt/trn_rl_repo",
    ):
        if os.path.isdir(p) and p not in sys.path:
            sys.path.append(p)


def build_nc():
    """Build the per-core Bass program (same program for all 8 cores)."""
    _ensure_paths()
    import concourse.mybir as mybir
    import concourse.tile as tile
    from concourse import bacc

    f32 = mybir.dt.float32
    f32r = mybir.dt.float32r
    Alu = mybir.AluOpType
    Act = mybir.ActivationFunctionType

    nc = bacc.Bacc("TRN2", target_bir_lowering=False, debug=False)

    nf = nc.declare_dram_parameter("nf", [128, 32, 128], f32, isOutput=False)
    smat = nc.declare_dram_parameter("smat", [128, 8], f32, isOutput=False)
    w1at = nc.declare_dram_parameter("w1at", [128, 2, 128], f32, isOutput=False)
    w1ct = nc.declare_dram_parameter("w1ct", [128, 2, 128], f32, isOutput=False)
    b1t = nc.declare_dram_parameter("b1t", [128, 2], f32, isOutput=False)
    w2b = nc.declare_dram_parameter("w2b", [128, 2, 32, 32], f32, isOutput=False)
    b2c = nc.declare_dram_parameter("b2c", [128, 1], f32, isOutput=False)
    outd = nc.declare_dram_parameter("out", [4, 32, 256], f32, isOutput=True)

    with tile.TileContext(nc) as tc:
        with (
            tc.tile_pool(name="const", bufs=1) as constp,
            tc.tile_pool(name="data", bufs=1) as datap,
            tc.tile_pool(name="act", bufs=12) as actp,
            tc.tile_pool(name="ph", bufs=1, space="PSUM") as php,
            tc.tile_pool(name="pc", bufs=1, space="PSUM") as pcp,
            tc.tile_pool(name="pl", bufs=4, space="PSUM") as plp,
        ):
            smat_sb = constp.tile([128, 8], f32, tag="smat")
            nc.sync.dma_start(out=smat_sb[:], in_=smat[:])
            w1at_sb = constp.tile([128, 2, 128], f32, tag="w1at")
            nc.sync.dma_start(out=w1at_sb[:], in_=w1at[:])
            w1ct_sb = constp.tile([128, 2, 128], f32, tag="w1ct")
            nc.sync.dma_start(out=w1ct_sb[:], in_=w1ct[:])
            b1t_sb = constp.tile([128, 2], f32, tag="b1t")
            nc.sync.dma_start(out=b1t_sb[:], in_=b1t[:])
            w2b_sb = constp.tile([128, 2, 32, 32], f32, tag="w2b")
            nc.sync.dma_start(out=w2b_sb[:], in_=w2b[:])
            w2br_sb = constp.tile([128, 2, 32, 32], f32r, tag="w2br")
            nc.vector.tensor_copy(w2br_sb[:], w2b_sb[:])
            b2c_sb = constp.tile([128, 1], f32, tag="b2c")
            nc.sync.dma_start(out=b2c_sb[:], in_=b2c[:])

            nf_sb = constp.tile([128, 32, 128], f32, tag="nf")
            nc.sync.dma_start(out=nf_sb[:], in_=nf[:])

            # hT[f, j] = mean over T, via per-octet matmuls against S.
            ph = php.tile([128, 256], f32, tag="ph")
            for o in range(32):
                nc.tensor.matmul(
                    ph[:, 8 * o : 8 * o + 8],
                    lhsT=nf_sb[:, o, :],
                    rhs=smat_sb[:],
                    start=True,
                    stop=True,
                )
            hT = datap.tile([128, 256], f32, tag="hT")
            nc.vector.tensor_copy(hT[:], ph[:])

            # cT[h, j] and aT[h, i]+b1 for both h-tiles.
            cT = [datap.tile([128, 256], f32, tag=f"cT{t}", name=f"cT{t}") for t in range(2)]
            aTb = [datap.tile([128, 128], f32, tag=f"aTb{t}", name=f"aTb{t}") for t in range(2)]
            for t in range(2):
                pc = pcp.tile([128, 256], f32, tag="pc")
                nc.tensor.matmul(
                    pc[:], lhsT=w1ct_sb[:, t, :], rhs=hT[:], start=True, stop=True
                )
                nc.scalar.copy(cT[t][:], pc[:])
                pa = pcp.tile([128, 128], f32, tag="pa")
                nc.tensor.matmul(
                    pa[:], lhsT=w1at_sb[:, t, :], rhs=hT[:, 0:128], start=True, stop=True
                )
                nc.vector.tensor_scalar(
                    aTb[t][:], pa[:], b1t_sb[:, t : t + 1], None, Alu.add
                )

            # Pairwise: accumulate each 32-i group into its own PSUM bank
            # (f32r matmuls only support col-group 0), drain groups as
            # they complete.
            pl = None
            for i in range(128):
                g, r = divmod(i, 32)
                if r == 0:
                    pl = plp.tile([32, 256], f32, tag="pl", name=f"pl{g}")
                for t in range(2):
                    idx = 2 * i + t
                    a_col = aTb[t][:, i : i + 1]
                    if idx % 3 != 2:
                        av = actp.tile([128, 256], f32r, tag="actv")
                        nc.vector.tensor_scalar(
                            av[:], cT[t][:], a_col, 0.0, Alu.add, Alu.max
                        )
                        mv = av
                    else:
                        asb = actp.tile([128, 256], f32r, tag="acts")
                        nc.scalar.activation(asb[:], cT[t][:], Act.Relu, bias=a_col)
                        mv = asb
                    nc.tensor.matmul(
                        pl[:, :],
                        lhsT=w2br_sb[:, t, r, :],
                        rhs=mv[:],
                        start=(r == 0 and t == 0),
                        stop=(r == 31 and t == 1),
                    )
                if r == 31:
                    osb = datap.tile([32, 256], f32, tag=f"osb{g}", name=f"osb{g}")
                    nc.vector.tensor_scalar(
                        osb[:], pl[:, :], b2c_sb[0:32, :], None, Alu.add
                    )
                    nc.sync.dma_start(out=outd[g], in_=osb[:])

    nc.compile()
    return nc


def make_in_maps(nodefeat, W1, b1, W2, b2):
    """Host-side sharding/layout prep (no arithmetic on tensor data)."""
    nodefeat = np.asarray(nodefeat, dtype=np.float32)
    W1 = np.asarray(W1, dtype=np.float32)
    b1 = np.asarray(b1, dtype=np.float32)
    W2 = np.asarray(W2, dtype=np.float32)
    b2 = np.asarray(b2, dtype=np.float32)

    smat = np.repeat(np.eye(8, dtype=np.float32), 16, axis=0) / np.float32(16.0)

    W1a, W1c = W1[:, :_F], W1[:, _F:]
    w1at = np.ascontiguousarray(np.stack([W1a[:128].T, W1a[128:].T], axis=1))
    w1ct = np.ascontiguousarray(np.stack([W1c[:128].T, W1c[128:].T], axis=1))
    b1t = np.ascontiguousarray(b1.reshape(2, 128).T)

    w2r = W2[0].reshape(2, 128)  # [ht, p]
    w2b = np.zeros((128, 2, 32, 32), dtype=np.float32)
    idx = np.arange(32)
    w2b[:, :, idx, idx] = w2r.T[:, :, None]

    b2c = np.full((128, 1), b2[0], dtype=np.float32)

    in_maps = []
    for k in range(_NCORES):
        b, ih = divmod(k, 2)
        nf_b = nodefeat[b]  # [256, 16, 128]
        if ih:
            nf_b = np.concatenate([nf_b[128:], nf_b[:128]], axis=0)
        # [256,16,128] -> [32 oct, (j8,t16)=128, 128 f] -> [128, 32, 128]
        nf_dev = np.ascontiguousarray(
            nf_b.reshape(32, 128, 128).transpose(1, 0, 2)
        )
        in_maps.append(
            {
                "nf": nf_dev,
                "smat": smat,
                "w1at": w1at,
                "w1ct": w1ct,
                "b1t": b1t,
                "w2b": w2b,
                "b2c": b2c,
            }
        )
    return in_maps


def assemble_output(results):
    out = np.empty((_B, _N, _N), dtype=np.float32)
    for k in range(_NCORES):
        b, ih = divmod(k, 2)
        r = results[k]["out"].reshape(128, 256)  # [i, j] (j core-local order)
        if ih:
            r = np.concatenate([r[:, 128:], r[:, :128]], axis=1)
        out[b, ih * 128 : (ih + 1) * 128, :] = r
    return out


def _get_nc():
    if "nc" not in _CACHE:
        _CACHE["nc"] = build_nc()
    return _CACHE["nc"]


def kernel(nodefeat, W1, b1, W2, b2):
    _ensure_paths()
    from concourse.bass_utils import run_bass_kernel_spmd

    nc = _get_nc()
    in_maps = make_in_maps(nodefeat, W1, b1, W2, b2)
    res = run_bass_kernel_spmd(nc, in_maps, list(range(_NCORES)))
    return assemble_output(res.results)
